# revision 22
# baseline (speedup 1.0000x reference)
"""AttnPool Trainium2 kernel (nn_AttnPool_73100343378373).

Math (algebraically identical to the reference):
    scores = (q @ w) @ x.T   per batch  -> (H, L)
    attn   = softmax(scores + mask_bias, axis=L)
    out    = attn @ x  -> (B, H*D)

Distribution: data-parallel over batch, 2 batches per core, qw replicated.

Precision scheme (validated vs fp64 reference: rel err 4.5e-4):
  - qw = q@w computed host-side in fp64, split into fp16 hi+lo planes,
    shipped pre-transposed. x shipped as a single fp16 plane.
  - scores = x16 @ (qw_hi + qw_lo)^T accumulated in fp32 PSUM; fp16*fp16
    products are exact in fp32. scoresT stays fp32 in SBUF (fp16 storage
    was measured at 1.6e-2 - too close to the 2e-2 gate).
  - attn weights u in fp16; pooled = u @ x16 accumulated in fp32.

vs the bf16 hi/lo baseline this halves HBM traffic (16MB/core), halves
the PE transposes and PSUM->SBUF copies, and cuts score matmul streams
3x (one moving stream per chunk with M=16 packed [qw_hi|qw_lo]
stationary).
"""

import os
from contextlib import ExitStack

import numpy as np

B, L, D, H = 16, 4096, 1024, 8
NCORES = 8
BPC = B // NCORES  # batches per core
NG = 8  # L-groups per batch
GL = L // NG  # rows per group = 512
NT = L // 128  # 128-row L-tiles per batch = 32
DC = D // 128  # 128-wide D chunks = 8

KD = 2  # D-chunks shipped pre-transposed from host (skip PE transpose)

VARIANT = {
    "xg_bufs": 2 * NG,       # all groups of both batches resident
    "xt_bufs": 4,
    "xtd_bufs": 3,
    "pst_bufs": 4,
    "exp_chunks": 8,
    "score_strips": 2,
    "pool_strips": 4,
}

_CACHE: dict = {}
LAST_RESULTS = None  # test harness can read exec_time_ns from here


def _build(masked: bool, variant: dict | None = None):
    import concourse.bass as bass
    import concourse.tile as tile
    from concourse import bacc, mybir
    from concourse.masks import make_identity

    v = dict(VARIANT)
    if variant:
        v.update(variant)

    f32 = mybir.dt.float32
    f16 = mybir.dt.float16
    AF = mybir.ActivationFunctionType
    AX = mybir.AxisListType
    ALU = mybir.AluOpType

    nc = bacc.Bacc("TRN2", target_bir_lowering=False, debug=False)

    x_d = nc.dram_tensor("x16", (BPC, L, D), f16, kind="ExternalInput").ap()
    # last KD chunks of x, host-transposed: [b, c, d-in-chunk, l]
    xT_d = nc.dram_tensor("xT16", (BPC, KD, 128, L), f16, kind="ExternalInput").ap()
    qwT_d = nc.dram_tensor("qwT", (D, H), f16, kind="ExternalInput").ap()
    if masked:
        mb_d = nc.dram_tensor("mb", (BPC, H, L), f32, kind="ExternalInput").ap()
    out_d = nc.dram_tensor("out", (BPC, H, D), f32, kind="ExternalOutput").ap()

    with tile.TileContext(nc) as tc, ExitStack() as ctx:
        const = ctx.enter_context(tc.tile_pool(name="const", bufs=1))
        xgp = ctx.enter_context(tc.tile_pool(name="xg", bufs=v["xg_bufs"]))
        xtp = ctx.enter_context(tc.tile_pool(name="xt", bufs=v["xt_bufs"]))
        xtdp = ctx.enter_context(tc.tile_pool(name="xtd", bufs=v["xtd_bufs"]))
        sbp = ctx.enter_context(tc.tile_pool(name="small", bufs=2))
        pst = ctx.enter_context(tc.tile_pool(name="pst", bufs=v["pst_bufs"], space="PSUM"))
        pss = ctx.enter_context(tc.tile_pool(name="pss", bufs=2, space="PSUM"))
        psp = ctx.enter_context(tc.tile_pool(name="psp", bufs=2, space="PSUM"))

        ident = const.tile([128, 128], f16, tag="ident")
        make_identity(nc, ident[:])

        # qwT chunks: [128 d-part, DC * H]; chunk j at cols Hj:Hj+H
        # (scalar queue: keeps gpsimd free to finish make_identity early)
        qwT_sb = const.tile([128, DC * H], f16, tag="qwT")
        nc.scalar.dma_start(
            qwT_sb[:].rearrange("p (c h) -> p c h", c=DC),
            qwT_d.rearrange("(c p) h -> p c h", p=128),
        )

        NSS = v["score_strips"]
        NPS = v["pool_strips"]

        xg_tiles = [[None] * NG for _ in range(BPC)]
        scoresT_t = [None] * BPC
        pmax_t = [None] * BPC
        mb_t = [None] * BPC

        # ---- phase 1 per batch: DMA + transposes + score matmuls
        def emit_groups(b):
            if masked:
                mb_sb = sbp.tile([H, L], f32, tag="mb")
                nc.gpsimd.dma_start(mb_sb[:], mb_d[b])
                mb_t[b] = mb_sb
            scoresT = sbp.tile([H, L], f32, tag="scoresT")
            scoresT_t[b] = scoresT
            pmax = sbp.tile([H, NG], f32, tag="pmax")
            pmax_t[b] = pmax
            for g in range(NG):
                xg = xgp.tile([128, 4 * D], f16, tag="xg", name="xg")
                nc.sync.dma_start(
                    xg[:].rearrange("p (t d) -> p t d", d=D),
                    x_d[b, GL * g : GL * (g + 1), :].rearrange(
                        "(t p) d -> p t d", p=128
                    ),
                )
                xg_tiles[b][g] = xg
                # host-transposed chunks DC-KD..DC-1 for this group's L cols
                xtd = xtdp.tile([128, KD * 512], f16, tag="xtd", name="xtd")
                nc.sync.dma_start(
                    xtd[:].rearrange("p (c l) -> p c l", c=KD),
                    xT_d[b, :, :, GL * g : GL * (g + 1)].rearrange(
                        "c p l -> p c l"
                    ),
                )
                sp = pss.tile([128, 512], f32, tag="pss")
                # all transposes + copies of the group first, then one
                # matmul burst: adjacent strip-alternating matmuls
                # overlap on the PE array (pairwise col-group concurrency)
                xts = []
                for jp in range((DC - KD) // 2):  # chunk pairs
                    ps = pst.tile([128, 1024], f16, tag="pst", name="xtps")
                    for k in range(2):
                        j = 2 * jp + k
                        for t in range(4):
                            nc.tensor.transpose(
                                ps[:, 512 * k + 128 * t : 512 * k + 128 * (t + 1)],
                                xg[:, D * t + 128 * j : D * t + 128 * (j + 1)],
                                ident[:],
                            )
                    xt = xtp.tile([128, 1024], f16, tag="xt", name="xt")
                    # 2:1 DVE:ACT split (DVE copies ~2x faster)
                    if (g * 3 + jp) % 3 == 2:
                        nc.scalar.copy(xt[:], ps[:])
                    else:
                        nc.vector.tensor_copy(xt[:], ps[:])
                    xts.append(xt)
                # burst order: DMA-fed chunks first (ready earliest)
                border = list(range(DC - KD, DC)) + list(range(DC - KD))
                for bi, j in enumerate(border):
                    s = bi % NSS
                    if j >= DC - KD:
                        src = xtd[:, 512 * (j - (DC - KD)) : 512 * (j - (DC - KD) + 1)]
                    else:
                        src = xts[j // 2][:, 512 * (j % 2) : 512 * (j % 2 + 1)]
                    nc.tensor.matmul(
                        sp[32 * s : 32 * s + H, :],
                        qwT_sb[:, H * j : H * (j + 1)],
                        src,
                        start=(bi < NSS),
                        stop=(bi >= DC - NSS),
                        tile_position=(0, 32 * s),
                        skip_group_check=True,
                    )
                # strip reduce
                sl = scoresT[:, GL * g : GL * (g + 1)]
                t1 = sbp.tile([H, 512], f32, tag="t1")
                nc.scalar.copy(t1[:], sp[0:H, :])
                if masked:
                    nc.vector.tensor_add(t1[:], t1[:], mb_t[b][:, GL * g : GL * (g + 1)])
                nc.vector.tensor_add(sl, t1[:], sp[32 : 32 + H, :])
                nc.vector.reduce_max(pmax[:, g : g + 1], sl, axis=AX.X)

        # ---- phase 2 per batch: softmax + uT + pooled matmuls + out
        def emit_tail(b):
            scoresT = scoresT_t[b]
            negmax = sbp.tile([H, 1], f32, tag="negmax")
            nc.vector.reduce_max(negmax[:], pmax_t[b][:], axis=AX.X, negate=True)
            u16 = sbp.tile([H, L], f16, tag="u16")
            NE = v["exp_chunks"]
            EW = L // NE
            sums = sbp.tile([H, NE], f32, tag="sums")
            for ch in range(NE):
                nc.scalar.activation(
                    u16[:, EW * ch : EW * (ch + 1)],
                    scoresT[:, EW * ch : EW * (ch + 1)],
                    AF.Exp,
                    bias=negmax[:],
                    scale=1.0,
                    accum_out=sums[:, ch : ch + 1],
                )
            stot = sbp.tile([H, 1], f32, tag="stot")
            nc.vector.reduce_sum(stot[:], sums[:], axis=AX.X)
            inv = sbp.tile([H, 1], f32, tag="inv")
            nc.vector.reciprocal(inv[:], stot[:])

            uT = sbp.tile([128, NT * H], f16, tag="uT")
            UB = 4  # l-tiles per psum tile
            for ib in range(NT // UB):
                ps = pst.tile([128, 1024], f16, tag="pst", name="utps")
                for kk in range(UB):
                    i = ib * UB + kk
                    nc.tensor.transpose(
                        ps[:, H * kk : H * (kk + 1)],
                        u16[:, 128 * i : 128 * (i + 1)],
                        ident[0:H, 0:H],
                    )
                dst = uT[:, H * UB * ib : H * UB * (ib + 1)]
                if ib % 2 == 0:
                    nc.vector.tensor_copy(dst, ps[:, 0 : H * UB])
                else:
                    nc.scalar.copy(dst, ps[:, 0 : H * UB])

            # hh-major so half 0's reduce/normalize/store overlaps half 1's
            # matmuls; strips pre-scaled by inv during the psum reduction
            pp = [psp.tile([128, 512], f32, tag="psp", name=f"pp{i}") for i in range(2)]
            pooled = sbp.tile([H, D], f32, tag="pooled")
            for hh in range(2):
                for i in range(NT):
                    g_, t_ = i // 4, i % 4
                    s = i % NPS
                    nc.tensor.matmul(
                        pp[hh][32 * s : 32 * s + H, :],
                        uT[:, H * i : H * (i + 1)],
                        xg_tiles[b][g_][:, D * t_ + 512 * hh : D * t_ + 512 * (hh + 1)],
                        start=(i < NPS),
                        stop=(i >= NT - NPS),
                        tile_position=(0, 32 * s),
                        skip_group_check=True,
                    )
                half = pooled[:, 512 * hh : 512 * (hh + 1)]
                th = sbp.tile([H, 512], f32, tag="th", name="th")
                nc.scalar.mul(th[:], pp[hh][0:H, :], inv[:])
                for s in range(1, NPS):
                    dst = half if s == NPS - 1 else th[:]
                    nc.vector.scalar_tensor_tensor(
                        out=dst,
                        in0=pp[hh][32 * s : 32 * s + H, :],
                        scalar=inv[:],
                        in1=th[:],
                        op0=ALU.mult,
                        op1=ALU.add,
                    )
                nc.scalar.dma_start(out_d[b][:, 512 * hh : 512 * (hh + 1)], half)

        for b in range(BPC):
            emit_groups(b)
        for b in range(BPC):
            emit_tail(b)

    nc.compile()
    return nc


def _get_nc(masked: bool):
    if masked not in _CACHE:
        _CACHE[masked] = _build(masked)
    return _CACHE[masked]


def make_in_maps(x, kpm, q, w, masked, variant=None):
    qw = q.astype(np.float64) @ w.astype(np.float64)  # (H, D)
    qwT = np.ascontiguousarray(qw.T.astype(np.float16))  # (D, H)
    x16 = np.asarray(x, np.float32).astype(np.float16)
    # last KD 128-wide D-chunks, transposed: (B, KD, 128, L)
    xT16 = np.ascontiguousarray(
        x16[:, :, (DC - KD) * 128 :]
        .reshape(B, L, KD, 128)
        .transpose(0, 2, 3, 1)
    )
    in_maps = []
    for c in range(NCORES):
        m = {
            "x16": np.ascontiguousarray(x16[BPC * c : BPC * (c + 1)]),
            "xT16": np.ascontiguousarray(xT16[BPC * c : BPC * (c + 1)]),
            "qwT": qwT,
        }
        if masked:
            bias = np.where(
                kpm[BPC * c : BPC * (c + 1), None, :], np.float32(-1e30), np.float32(0)
            ).astype(np.float32)
            m["mb"] = np.ascontiguousarray(np.broadcast_to(bias, (BPC, H, L)))
        in_maps.append(m)
    return in_maps


def kernel(**inputs) -> np.ndarray:
    global LAST_RESULTS
    from concourse.bass_utils import run_bass_kernel_spmd

    x = np.asarray(inputs["x"], dtype=np.float32)
    kpm = np.asarray(inputs["kpm"])
    q = np.asarray(inputs["q"], dtype=np.float32)
    w = np.asarray(inputs["w"], dtype=np.float32)

    masked = bool(kpm.any())
    nc = _get_nc(masked)
    in_maps = make_in_maps(x, kpm, q, w, masked)

    trace = bool(os.environ.get("ATTNPOOL_TRACE"))
    res = run_bass_kernel_spmd(nc, in_maps, list(range(NCORES)), trace=trace)
    LAST_RESULTS = res
    out = np.concatenate(
        [r["out"].reshape(BPC, H * D) for r in res.results], axis=0
    )
    return np.ascontiguousarray(out.astype(np.float32))


# revision 29
# speedup vs baseline: 1.0400x; 1.0400x over previous
"""AttnPool Trainium2 kernel (nn_AttnPool_73100343378373).

Math (algebraically identical to the reference):
    scores = (q @ w) @ x.T   per batch  -> (H, L)
    attn   = softmax(scores + mask_bias, axis=L)
    out    = attn @ x  -> (B, H*D)

Distribution: data-parallel over batch, 2 batches per core, qw replicated.

Precision scheme (validated vs fp64 reference: rel err 4.5e-4):
  - qw = q@w computed host-side in fp64, split into fp16 hi+lo planes,
    shipped pre-transposed. x shipped as a single fp16 plane.
  - scores = x16 @ (qw_hi + qw_lo)^T accumulated in fp32 PSUM; fp16*fp16
    products are exact in fp32. scoresT stays fp32 in SBUF (fp16 storage
    was measured at 1.6e-2 - too close to the 2e-2 gate).
  - attn weights u in fp16; pooled = u @ x16 accumulated in fp32.

vs the bf16 hi/lo baseline this halves HBM traffic (16MB/core), halves
the PE transposes and PSUM->SBUF copies, and cuts score matmul streams
3x (one moving stream per chunk with M=16 packed [qw_hi|qw_lo]
stationary).
"""

import os
from contextlib import ExitStack

import numpy as np

B, L, D, H = 16, 4096, 1024, 8
NCORES = 8
BPC = B // NCORES  # batches per core
NG = 8  # L-groups per batch
GL = L // NG  # rows per group = 512
NT = L // 128  # 128-row L-tiles per batch = 32
DC = D // 128  # 128-wide D chunks = 8

KD = 2  # D-chunks shipped pre-transposed from host (skip PE transpose)

VARIANT = {
    "xg_bufs": 2 * NG,       # all groups of both batches resident
    "xt_bufs": 4,
    "xtd_bufs": 3,
    "pst_bufs": 3,
    "exp_chunks": 8,
    "score_strips": 2,
    "pool_strips": 4,
}

_CACHE: dict = {}
LAST_RESULTS = None  # test harness can read exec_time_ns from here


def _build(masked: bool, variant: dict | None = None):
    import concourse.bass as bass
    import concourse.tile as tile
    from concourse import bacc, mybir
    from concourse.masks import make_identity

    v = dict(VARIANT)
    if variant:
        v.update(variant)

    f32 = mybir.dt.float32
    f16 = mybir.dt.float16
    AF = mybir.ActivationFunctionType
    AX = mybir.AxisListType
    ALU = mybir.AluOpType

    nc = bacc.Bacc("TRN2", target_bir_lowering=False, debug=False)

    x_d = nc.dram_tensor("x16", (BPC, L, D), f16, kind="ExternalInput").ap()
    # last KD chunks of x, host-transposed: [b, c, d-in-chunk, l]
    xT_d = nc.dram_tensor("xT16", (BPC, KD, 128, L), f16, kind="ExternalInput").ap()
    qwT_d = nc.dram_tensor("qwT", (D, H), f16, kind="ExternalInput").ap()
    if masked:
        mb_d = nc.dram_tensor("mb", (BPC, H, L), f32, kind="ExternalInput").ap()
    out_d = nc.dram_tensor("out", (BPC, H, D), f32, kind="ExternalOutput").ap()

    with tile.TileContext(nc) as tc, ExitStack() as ctx:
        const = ctx.enter_context(tc.tile_pool(name="const", bufs=1))
        xgp = ctx.enter_context(tc.tile_pool(name="xg", bufs=v["xg_bufs"]))
        xtp = ctx.enter_context(tc.tile_pool(name="xt", bufs=v["xt_bufs"]))
        xtdp = ctx.enter_context(tc.tile_pool(name="xtd", bufs=v["xtd_bufs"]))
        sbp = ctx.enter_context(tc.tile_pool(name="small", bufs=2))
        pst = ctx.enter_context(tc.tile_pool(name="pst", bufs=v["pst_bufs"], space="PSUM"))
        pss = ctx.enter_context(tc.tile_pool(name="pss", bufs=3, space="PSUM"))
        psp = ctx.enter_context(tc.tile_pool(name="psp", bufs=2, space="PSUM"))

        ident = const.tile([128, 128], f16, tag="ident")
        make_identity(nc, ident[:])

        # qwT chunks: [128 d-part, DC * H]; chunk j at cols Hj:Hj+H
        # (scalar queue: keeps gpsimd free to finish make_identity early)
        qwT_sb = const.tile([128, DC * H], f16, tag="qwT")
        nc.scalar.dma_start(
            qwT_sb[:].rearrange("p (c h) -> p c h", c=DC),
            qwT_d.rearrange("(c p) h -> p c h", p=128),
        )

        NSS = v["score_strips"]
        NPS = v["pool_strips"]

        xg_tiles = [[None] * NG for _ in range(BPC)]
        scoresT_t = [None] * BPC
        pmax_t = [None] * BPC
        mb_t = [None] * BPC
        uT_t = [None] * BPC
        inv_t = [None] * BPC

        # ---- phase 1 per batch: DMA + transposes + score matmuls
        # (generator: yields after each of the NG groups)
        def emit_groups(b):
            if masked:
                mb_sb = sbp.tile([H, L], f32, tag="mb")
                nc.gpsimd.dma_start(mb_sb[:], mb_d[b])
                mb_t[b] = mb_sb
            scoresT = sbp.tile([H, L], f32, tag="scoresT")
            scoresT_t[b] = scoresT
            pmax = sbp.tile([H, NG], f32, tag="pmax")
            pmax_t[b] = pmax
            for g in range(NG):
                xg = xgp.tile([128, 4 * D], f16, tag="xg", name="xg")
                nc.sync.dma_start(
                    xg[:].rearrange("p (t d) -> p t d", d=D),
                    x_d[b, GL * g : GL * (g + 1), :].rearrange(
                        "(t p) d -> p t d", p=128
                    ),
                )
                xg_tiles[b][g] = xg
                # host-transposed chunks DC-KD..DC-1 for this group's L cols
                xtd = xtdp.tile([128, KD * 512], f16, tag="xtd", name="xtd")
                nc.sync.dma_start(
                    xtd[:].rearrange("p (c l) -> p c l", c=KD),
                    xT_d[b, :, :, GL * g : GL * (g + 1)].rearrange(
                        "c p l -> p c l"
                    ),
                )
                sp = pss.tile([128, 512], f32, tag="pss")
                # all transposes + copies of the group first, then one
                # matmul burst: adjacent strip-alternating matmuls
                # overlap on the PE array (pairwise col-group concurrency)
                xts = []
                for jp in range((DC - KD) // 2):  # chunk pairs
                    ps = pst.tile([128, 1024], f16, tag="pst", name="xtps")
                    for k in range(2):
                        j = 2 * jp + k
                        for t in range(4):
                            nc.tensor.transpose(
                                ps[:, 512 * k + 128 * t : 512 * k + 128 * (t + 1)],
                                xg[:, D * t + 128 * j : D * t + 128 * (j + 1)],
                                ident[:],
                            )
                    xt = xtp.tile([128, 1024], f16, tag="xt", name="xt")
                    # 2:1 DVE:ACT split (DVE copies ~2x faster)
                    if (g * 3 + jp) % 3 == 2:
                        nc.scalar.copy(xt[:], ps[:])
                    else:
                        nc.vector.tensor_copy(xt[:], ps[:])
                    xts.append(xt)
                # burst order: DMA-fed chunks first (ready earliest)
                border = list(range(DC - KD, DC)) + list(range(DC - KD))
                for bi, j in enumerate(border):
                    s = bi % NSS
                    if j >= DC - KD:
                        src = xtd[:, 512 * (j - (DC - KD)) : 512 * (j - (DC - KD) + 1)]
                    else:
                        src = xts[j // 2][:, 512 * (j % 2) : 512 * (j % 2 + 1)]
                    nc.tensor.matmul(
                        sp[32 * s : 32 * s + H, :],
                        qwT_sb[:, H * j : H * (j + 1)],
                        src,
                        start=(bi < NSS),
                        stop=(bi >= DC - NSS),
                        tile_position=(0, 32 * s),
                        skip_group_check=True,
                    )
                # strip reduce
                sl = scoresT[:, GL * g : GL * (g + 1)]
                t1 = sbp.tile([H, 512], f32, tag="t1")
                nc.scalar.copy(t1[:], sp[0:H, :])
                if masked:
                    nc.vector.tensor_add(t1[:], t1[:], mb_t[b][:, GL * g : GL * (g + 1)])
                nc.vector.tensor_add(sl, t1[:], sp[32 : 32 + H, :])
                nc.vector.reduce_max(pmax[:, g : g + 1], sl, axis=AX.X)
                yield

        # ---- phase 2a: softmax + uT, generator yielding once per L-chunk
        # so it can be woven into another batch's instruction stream
        def emit_softmax(b):
            scoresT = scoresT_t[b]
            negmax = sbp.tile([H, 1], f32, tag="negmax")
            nc.vector.reduce_max(negmax[:], pmax_t[b][:], axis=AX.X, negate=True)
            u16 = sbp.tile([H, L], f16, tag="u16")
            NE = v["exp_chunks"]
            EW = L // NE
            sums = sbp.tile([H, NE], f32, tag="sums")
            uT = sbp.tile([128, NT * H], f16, tag="uT")
            uT_t[b] = uT
            UB = 4  # l-tiles per uT psum tile (= one 512-col exp chunk)
            for ch in range(NE):
                nc.scalar.activation(
                    u16[:, EW * ch : EW * (ch + 1)],
                    scoresT[:, EW * ch : EW * (ch + 1)],
                    AF.Exp,
                    bias=negmax[:],
                    scale=1.0,
                    accum_out=sums[:, ch : ch + 1],
                )
                ps = pst.tile([128, 1024], f16, tag="pst", name="utps")
                for kk in range(UB):
                    i = ch * UB + kk
                    nc.tensor.transpose(
                        ps[:, H * kk : H * (kk + 1)],
                        u16[:, 128 * i : 128 * (i + 1)],
                        ident[0:H, 0:H],
                    )
                dst = uT[:, H * UB * ch : H * UB * (ch + 1)]
                if ch % 2 == 0:
                    nc.vector.tensor_copy(dst, ps[:, 0 : H * UB])
                else:
                    nc.scalar.copy(dst, ps[:, 0 : H * UB])
                yield
            stot = sbp.tile([H, 1], f32, tag="stot")
            nc.vector.reduce_sum(stot[:], sums[:], axis=AX.X)
            inv = sbp.tile([H, 1], f32, tag="inv")
            nc.vector.reciprocal(inv[:], stot[:])
            inv_t[b] = inv

        # ---- phase 2b: pooled matmuls + reduce + store, yields per i-chunk.
        # hh-major so half 0's reduce/normalize/store overlaps half 1's MMs;
        # strips are pre-scaled by 1/sum during the psum reduction
        def emit_pooled(b):
            uT = uT_t[b]
            inv = inv_t[b]
            pp = [psp.tile([128, 512], f32, tag="psp", name=f"pp{i}") for i in range(2)]
            pooled = sbp.tile([H, D], f32, tag="pooled")
            for hh in range(2):
                for ib in range(NT // 4):
                    for kk in range(4):
                        i = ib * 4 + kk
                        g_, t_ = i // 4, i % 4
                        s = i % NPS
                        nc.tensor.matmul(
                            pp[hh][32 * s : 32 * s + H, :],
                            uT[:, H * i : H * (i + 1)],
                            xg_tiles[b][g_][
                                :, D * t_ + 512 * hh : D * t_ + 512 * (hh + 1)
                            ],
                            start=(i < NPS),
                            stop=(i >= NT - NPS),
                            tile_position=(0, 32 * s),
                            skip_group_check=True,
                        )
                    yield
                half = pooled[:, 512 * hh : 512 * (hh + 1)]
                th = sbp.tile([H, 512], f32, tag="th", name="th")
                nc.scalar.mul(th[:], pp[hh][0:H, :], inv[:])
                for s in range(1, NPS):
                    dst = half if s == NPS - 1 else th[:]
                    nc.vector.scalar_tensor_tensor(
                        out=dst,
                        in0=pp[hh][32 * s : 32 * s + H, :],
                        scalar=inv[:],
                        in1=th[:],
                        op0=ALU.mult,
                        op1=ALU.add,
                    )
                nc.scalar.dma_start(out_d[b][:, 512 * hh : 512 * (hh + 1)], half)

        # ---- software-pipelined emission: engines execute their FIFOs in
        # order, so weave batch 0's softmax into batch 1's group stream and
        # batch 1's softmax into batch 0's pooled stream
        for _ in emit_groups(0):
            pass
        sm0 = emit_softmax(0)
        for _ in emit_groups(1):
            next(sm0, None)
        for _ in sm0:
            pass
        p0 = emit_pooled(0)
        sm1 = emit_softmax(1)
        for k, _ in enumerate(p0):
            if k % 2 == 1:
                next(sm1, None)
        for _ in sm1:
            pass
        for _ in emit_pooled(1):
            pass

    nc.compile()
    return nc


def _get_nc(masked: bool):
    if masked not in _CACHE:
        _CACHE[masked] = _build(masked)
    return _CACHE[masked]


def make_in_maps(x, kpm, q, w, masked, variant=None):
    qw = q.astype(np.float64) @ w.astype(np.float64)  # (H, D)
    qwT = np.ascontiguousarray(qw.T.astype(np.float16))  # (D, H)
    x16 = np.asarray(x, np.float32).astype(np.float16)
    # last KD 128-wide D-chunks, transposed: (B, KD, 128, L)
    xT16 = np.ascontiguousarray(
        x16[:, :, (DC - KD) * 128 :]
        .reshape(B, L, KD, 128)
        .transpose(0, 2, 3, 1)
    )
    in_maps = []
    for c in range(NCORES):
        m = {
            "x16": np.ascontiguousarray(x16[BPC * c : BPC * (c + 1)]),
            "xT16": np.ascontiguousarray(xT16[BPC * c : BPC * (c + 1)]),
            "qwT": qwT,
        }
        if masked:
            bias = np.where(
                kpm[BPC * c : BPC * (c + 1), None, :], np.float32(-1e30), np.float32(0)
            ).astype(np.float32)
            m["mb"] = np.ascontiguousarray(np.broadcast_to(bias, (BPC, H, L)))
        in_maps.append(m)
    return in_maps


def kernel(**inputs) -> np.ndarray:
    global LAST_RESULTS
    from concourse.bass_utils import run_bass_kernel_spmd

    x = np.asarray(inputs["x"], dtype=np.float32)
    kpm = np.asarray(inputs["kpm"])
    q = np.asarray(inputs["q"], dtype=np.float32)
    w = np.asarray(inputs["w"], dtype=np.float32)

    masked = bool(kpm.any())
    nc = _get_nc(masked)
    in_maps = make_in_maps(x, kpm, q, w, masked)

    trace = bool(os.environ.get("ATTNPOOL_TRACE"))
    res = run_bass_kernel_spmd(nc, in_maps, list(range(NCORES)), trace=trace)
    LAST_RESULTS = res
    out = np.concatenate(
        [r["out"].reshape(BPC, H * D) for r in res.results], axis=0
    )
    return np.ascontiguousarray(out.astype(np.float32))
